# revision 1
# baseline (speedup 1.0000x reference)
# Bass/Tile kernel for nn_LongTermAttention (continuous long-term attention
# with rectangular basis functions) on 8 Trainium2 NeuronCores.
#
# Mathematical rewrite (verified exact vs the reference):
#   * G = F^T (F F^T + ridge I)^{-1} for the rectangular basis on the padded
#     uniform grid collapses to G[l, n] = (1/4.5) * [l // 4 == n], so
#     Bc[b,n,e] = (1/4.5) * sum_{j<4} k[b,e,4n+j]  (4-wide sum pooling).
#   * psi on the integration grid is a one-hot selector: for p < 999,
#     psi[n, p] = [n == floor(512 p / 999)]; column p=999 is all zero.
#     Hence scores[b,h,t,p] = s[b,h,t,n(p)] (piecewise constant) and the
#     P=1000-point continuous softmax reduces to basis space:
#       u_n   = exp(s_n + log Wn_n)          (Wn = quadrature mass per basis)
#       Z     = sum_n u_n + w_last           (w_last from the psi==0 column)
#       ctx   = (u / Z) @ values
#     The max-subtraction in the reference cancels exactly (scores are O(3),
#     exp is safe unstabilized).
#
# Sharding: data-parallel over batch, 2 batches per core; weights replicated.

import numpy as np

B_FULL = 16
N_CORES = 8
B_PER = B_FULL // N_CORES  # 2
E = 512          # embed dim
L = 2048         # memory length
T = 256          # query length
N = 512          # basis count
H = 8            # heads
D = 64           # head dim
P_GRID = 1000    # integration points
RIDGE_C = 4.5    # F F^T diag (4.0) + ridge (0.5)

_CACHE = {}


def _host_constants(Wk, Wv):
    """Fold pooling normalization (1/4.5) and query scale (1/8) into the
    projection weights; build log quadrature-mass vector."""
    import ml_dtypes
    wk = (Wk.astype(np.float64) / (RIDGE_C * 8.0)).astype(ml_dtypes.bfloat16)
    wv = (Wv.astype(np.float64) / RIDGE_C).astype(ml_dtypes.bfloat16)
    p = np.arange(P_GRID)
    nmap = (512 * p) // 999
    w = np.full(P_GRID, 1.0 / 999.0)
    w[0] = w[-1] = 1.0 / 1998.0
    Wn = np.zeros(N)
    for i in range(P_GRID - 1):
        Wn[nmap[i]] += w[i]
    lnw = np.ascontiguousarray(np.log(Wn).astype(np.float32).reshape(4, 128).T)
    w_last = float(w[-1])
    return wk, wv, lnw, w_last


def _build_program(w_last):
    import concourse.bass as bass
    import concourse.mybir as mybir
    import concourse.tile as tile
    from concourse import bacc
    from concourse.masks import make_identity

    f32 = mybir.dt.float32
    bf16 = mybir.dt.bfloat16

    nc = bacc.Bacc(
        "TRN2",
        target_bir_lowering=False,
        debug=False,
        enable_asserts=False,
        num_devices=N_CORES,
    )

    k_d = nc.dram_tensor("k", [B_PER, E, L], bf16, kind="ExternalInput").ap()
    q_d = nc.dram_tensor("q", [B_PER, T, E], bf16, kind="ExternalInput").ap()
    wk_d = nc.dram_tensor("wk", [E, E], bf16, kind="ExternalInput").ap()
    wv_d = nc.dram_tensor("wv", [E, E], bf16, kind="ExternalInput").ap()
    lnw_d = nc.dram_tensor("lnw", [128, 4], f32, kind="ExternalInput").ap()
    out_d = nc.dram_tensor("out", [B_PER, T, E], f32, kind="ExternalOutput").ap()

    from contextlib import ExitStack
    with tile.TileContext(nc) as tc, ExitStack() as ctx:
        _kernel_body(ctx, tc, nc, mybir, make_identity,
                     k_d, q_d, wk_d, wv_d, lnw_d, out_d, w_last)

    nc.compile()
    return nc


def _kernel_body(ctx, tc, nc, mybir, make_identity,
                 k_d, q_d, wk_d, wv_d, lnw_d, out_d, w_last):
    f32 = mybir.dt.float32
    bf16 = mybir.dt.bfloat16
    Exp = mybir.ActivationFunctionType.Exp

    def pool(name, bufs, space="SBUF"):
        return ctx.enter_context(tc.tile_pool(name=name, bufs=bufs, space=space))

    consts = pool("consts", 1)
    kpool = pool("kpool", 6)
    t1pool = pool("t1pool", 4)
    plpool = pool("plpool", 8)
    ktpool = pool("ktpool", 8)
    vpool = pool("vpool", 8)
    qpool = pool("qpool", 3)
    qtpool = pool("qtpool", 8)
    upool = pool("upool", 16)
    rzpool = pool("rzpool", 8)
    opool = pool("opool", 4)

    ps_proj = pool("ps_proj", 2, "PSUM")   # [128,512] tiles: 2 banks (also qT)
    ps_s = pool("ps_s", 2, "PSUM")         # [128,1024] tiles: 4 banks
    ps_c = pool("ps_c", 2, "PSUM")         # [128,65] tiles: 2 banks

    # ---- small constants first (they gate the projections / scores) ----
    wk_sb = consts.tile([128, 4 * 512], bf16, tag="wk")   # [e%128, (e//128)*512 + e']
    wv_sb = consts.tile([128, 4 * 512], bf16, tag="wv")
    nc.sync.dma_start(wk_sb[:].rearrange("p (kk e) -> p kk e", kk=4),
                      wk_d.rearrange("(kk p) e -> p kk e", p=128))
    nc.scalar.dma_start(wv_sb[:].rearrange("p (kk e) -> p kk e", kk=4),
                        wv_d.rearrange("(kk p) e -> p kk e", p=128))
    lnw_sb = consts.tile([128, 4], f32, tag="lnw")
    nc.sync.dma_start(lnw_sb[:], lnw_d[:])

    # ---- k DMA + pooling for BOTH batches (split over rings + engines),
    #      chunked at half-tiles so pooling starts as soon as data lands ----
    pooled_b = []
    for b in range(B_PER):
        pooled = []
        for et in range(4):
            pl = plpool.tile([128, N], bf16, tag="pl")
            eng = nc.vector if et % 2 == 0 else nc.gpsimd
            dma_eng = nc.sync if et % 2 == 0 else nc.scalar
            kt = kpool.tile([128, L], bf16, tag="k")
            dma_eng.dma_start(kt[:], k_d[b, et * 128:(et + 1) * 128, :])
            t1 = t1pool.tile([128, L // 2], f32,
                             tag="t1v" if et % 2 == 0 else "t1g")
            kv = kt[:].rearrange("p (i two) -> p i two", two=2)
            eng.tensor_add(t1[:], kv[:, :, 0], kv[:, :, 1])
            tv = t1[:].rearrange("p (i two) -> p i two", two=2)
            eng.tensor_add(pl[:], tv[:, :, 0], tv[:, :, 1])
            pooled.append(pl)
        pooled_b.append(pooled)

    # ---- qT via DMA xbar transpose on the scalar ring ----
    qT_b = []
    for b in range(B_PER):
        qT = []
        for eb in range(4):
            qt_sb = qtpool.tile([128, T], bf16, tag="qT")
            nc.sync.dma_start(qt_sb[:], q_d[b, :, eb * 128:(eb + 1) * 128],
                              transpose=True)
            qT.append(qt_sb)
        qT_b.append(qT)

    for b in range(B_PER):
        pooled = pooled_b[b]
        qT = qT_b[b]
        # ---- keysT = wk^T @ pooled  -> [e' (4x128 part), n=512] ----
        keysT = []
        for m in range(4):
            ps = ps_proj.tile([128, 512], f32, tag="ps_proj")
            for kk in range(4):
                nc.tensor.matmul(
                    ps[:],
                    wk_sb[:, kk * 512 + m * 128: kk * 512 + (m + 1) * 128],
                    pooled[kk][:],
                    start=(kk == 0), stop=(kk == 3),
                )
            kt_sb = ktpool.tile([128, 512], bf16, tag="keysT")
            nc.scalar.copy(kt_sb[:], ps[:])
            keysT.append(kt_sb)

        # ---- values = pooled^T @ wv -> [n (4x128 part), e'=512],
        #      stored head-interleaved with a ones column: [n, 8*(64+1)] ----
        values = []
        for m in range(4):
            ps = ps_proj.tile([128, 512], f32, tag="ps_proj")
            for kk in range(4):
                nc.tensor.matmul(
                    ps[:],
                    pooled[kk][:, m * 128:(m + 1) * 128],
                    wv_sb[:, kk * 512:(kk + 1) * 512],
                    start=(kk == 0), stop=(kk == 3),
                )
            v_sb = vpool.tile([128, 8 * 66], bf16, tag="values")
            vv = v_sb[:].rearrange("p (h c) -> p h c", c=66)
            nc.vector.tensor_copy(
                vv[:, :, 0:64],
                ps[:].rearrange("p (h d) -> p h d", d=64),
            )
            nc.vector.memset(vv[:, :, 64], 1.0)
            values.append(v_sb)

        # ---- per head pair: scores -> exp -> u -> ctx ----
        # Score pair goes to a 2-bank PSUM tile: head 2hp+h01 in bank h01
        # (cols h01*512 .. h01*512+256) so the two row-packed matmuls never
        # drain into the same bank concurrently.
        out_sbs = [opool.tile([128, E], f32, tag="out", name=f"out{b}_{mb}")
                   for mb in range(2)]
        for hp in range(4):
            if hp == 2:
                for mb in range(2):
                    nc.sync.dma_start(out_d[b, mb * 128:(mb + 1) * 128, 0:256],
                                      out_sbs[mb][:, 0:256])
            u_tiles = {}
            for nb in range(4):
                ps = ps_s.tile([128, 1024], f32, tag="ps_s")
                for h01 in range(2):
                    nc.tensor.matmul(
                        ps[:, h01 * 512: h01 * 512 + 256],
                        keysT[hp][h01 * 64:(h01 + 1) * 64,
                                  nb * 128:(nb + 1) * 128],
                        qT[hp][h01 * 64:(h01 + 1) * 64, :],
                        start=True, stop=True,
                        skip_group_check=True,
                    )
                u = upool.tile([128, 512], bf16, tag="u")
                nc.scalar.activation(
                    u[:].rearrange("p (g c) -> p g c", c=256),
                    ps[:].rearrange("p (g c) -> p g c", c=512)[:, :, 0:256],
                    Exp, bias=lnw_sb[:, nb:nb + 1], scale=1.0)
                u_tiles[nb] = u

            # ctx (+Z in col 64): one PSUM bank per (head, t-block) chain
            for h01 in range(2):
                h = hp * 2 + h01
                for mb in range(2):
                    ps = ps_c.tile([128, 65], f32, tag="ps_c")
                    for nb in range(4):
                        nc.tensor.matmul(
                            ps[:],
                            u_tiles[nb][:, h01 * 256 + mb * 128:
                                        h01 * 256 + (mb + 1) * 128],
                            values[nb][:, h * 66:h * 66 + 65],
                            start=(nb == 0), stop=(nb == 3),
                        )
                    rz = rzpool.tile([128, 1], f32, tag="rz")
                    nc.vector.tensor_scalar_add(rz[:], ps[:, 64:65], w_last)
                    rzi = rzpool.tile([128, 1], f32, tag="rzi")
                    nc.vector.reciprocal(rzi[:], rz[:])
                    nc.vector.tensor_scalar_mul(
                        out_sbs[mb][:, h * 64:(h + 1) * 64],
                        ps[:, 0:64], rzi[:])
        for mb in range(2):
            nc.sync.dma_start(out_d[b, mb * 128:(mb + 1) * 128, 256:512],
                              out_sbs[mb][:, 256:512])


def _get_program(w_last):
    if "nc" not in _CACHE:
        _CACHE["nc"] = _build_program(w_last)
    return _CACHE["nc"]


def make_in_maps(k, q, Wk, Wv):
    import ml_dtypes
    wk, wv, lnw, w_last = _host_constants(Wk, Wv)
    k16 = np.asarray(k).astype(ml_dtypes.bfloat16)
    q16 = np.asarray(q).astype(ml_dtypes.bfloat16)
    in_maps = []
    for c in range(N_CORES):
        in_maps.append({
            "k": np.ascontiguousarray(k16[c * B_PER:(c + 1) * B_PER]),
            "q": np.ascontiguousarray(q16[c * B_PER:(c + 1) * B_PER]),
            "wk": wk,
            "wv": wv,
            "lnw": lnw,
        })
    return in_maps, w_last


def kernel(k, q, Wk, Wv):
    from concourse.bass_utils import run_bass_kernel_spmd

    in_maps, w_last = make_in_maps(k, q, Wk, Wv)
    nc = _get_program(w_last)
    res = run_bass_kernel_spmd(nc, in_maps, core_ids=list(range(N_CORES)))
    return np.concatenate([res.results[c]["out"] for c in range(N_CORES)], axis=0)



# revision 6
# speedup vs baseline: 1.0116x; 1.0116x over previous
# Bass/Tile kernel for nn_LongTermAttention (continuous long-term attention
# with rectangular basis functions) on 8 Trainium2 NeuronCores.
#
# Mathematical rewrite (verified exact vs the reference):
#   * G = F^T (F F^T + ridge I)^{-1} for the rectangular basis on the padded
#     uniform grid collapses to G[l, n] = (1/4.5) * [l // 4 == n], so
#     Bc[b,n,e] = (1/4.5) * sum_{j<4} k[b,e,4n+j]  (4-wide sum pooling).
#   * psi on the integration grid is a one-hot selector, so the P=1000-point
#     continuous softmax reduces to basis space with quadrature mass Wn per
#     basis:  p_n = exp(s_n) Wn / Z,  Z = sum_n exp(s_n) Wn + w_last.
#     Here Wn is folded into the VALUES tiles (and the Z "ones" column), so
#     the device only computes a bias-free exp.
#   * The max-subtraction in the reference cancels exactly.
#
# Performance structure (v2):
#   * Host-side layout permutes (free): k -> [b, et, p, j, n] so pooling is
#     two contiguous bf16 adds; q pre-transposed; weights pre-laid-out.
#   * Two HW DMA rings (sync + act) with priority-ordered queues.
#   * PE warm-up matmuls (zeros) to ramp the tensor-engine p-state before
#     real data lands, then an emission order that keeps the PE busy:
#     b0 proj -> b0 scores (woven with b1 proj) -> b1 scores (woven with
#     b0 ctx) -> b1 ctx.
#   * exp as 16 big [128,1024] scalar activations, pipelined under the PE.
#
# Sharding: data-parallel over batch, 2 batches per core; weights replicated.

import numpy as np

B_FULL = 16
N_CORES = 8
B_PER = B_FULL // N_CORES  # 2
E = 512          # embed dim
L = 2048         # memory length
T = 256          # query length
N = 512          # basis count
H = 8            # heads
D = 64           # head dim
P_GRID = 1000    # integration points
RIDGE_C = 4.5    # F F^T diag (4.0) + ridge (0.5)

N_WARM1 = 7      # junk matmuls before first real matmul
N_WARM2 = 7      # junk matmuls inside the first keysT chain gap

_CACHE = {}


def _host_constants(Wk, Wv):
    """Pre-scale and lay out weights; build quadrature-mass vectors."""
    import ml_dtypes
    bf16 = ml_dtypes.bfloat16
    # fold pooling normalization (1/4.5) and query scale (1/8) into Wk;
    # layout [p, kk, e'] with e = kk*128 + p
    wk = (Wk.astype(np.float64) / (RIDGE_C * 8.0)).astype(np.float32)
    wv = (Wv.astype(np.float64) / RIDGE_C).astype(np.float32)
    wk_l = np.ascontiguousarray(
        wk.reshape(4, 128, 512).transpose(1, 0, 2)).astype(bf16)
    wv_l = np.ascontiguousarray(
        wv.reshape(4, 128, 512).transpose(1, 0, 2)).astype(bf16)
    # quadrature mass per basis (trapezoid weights summed per bin, p<999)
    p = np.arange(P_GRID)
    nmap = (512 * p) // 999
    w = np.full(P_GRID, 1.0 / 999.0)
    w[0] = w[-1] = 1.0 / 1998.0
    Wn = np.zeros(N)
    for i in range(P_GRID - 1):
        Wn[nmap[i]] += w[i]
    # wn8[p, ms, h] = Wn[ms*128 + p] replicated over 8 heads
    wn8 = np.ascontiguousarray(
        np.repeat(Wn.reshape(4, 128).T[:, :, None], H, axis=2)
    ).astype(np.float32)
    w_last = float(w[-1])
    return wk_l, wv_l, wn8, w_last


def _build_program(w_last):
    import concourse.bass as bass
    import concourse.mybir as mybir
    import concourse.tile as tile
    from concourse import bacc

    nc = bacc.Bacc(
        "TRN2",
        target_bir_lowering=False,
        debug=False,
        enable_asserts=False,
        num_devices=N_CORES,
    )

    f32 = mybir.dt.float32
    bf16 = mybir.dt.bfloat16

    k_d = nc.dram_tensor("k", [B_PER, 4, 128, 4, N], bf16,
                         kind="ExternalInput").ap()
    q_d = nc.dram_tensor("q", [B_PER, 128, 4, T], bf16,
                         kind="ExternalInput").ap()
    wk_d = nc.dram_tensor("wk", [128, 4, E], bf16, kind="ExternalInput").ap()
    wv_d = nc.dram_tensor("wv", [128, 4, E], bf16, kind="ExternalInput").ap()
    wn8_d = nc.dram_tensor("wn8", [128, 4, H], f32, kind="ExternalInput").ap()
    out_d = nc.dram_tensor("out", [B_PER, T, E], f32,
                           kind="ExternalOutput").ap()

    from contextlib import ExitStack
    with tile.TileContext(nc) as tc, ExitStack() as ctx:
        _kernel_body(ctx, tc, nc, mybir, k_d, q_d, wk_d, wv_d, wn8_d, out_d,
                     w_last)

    nc.compile()
    return nc


def _kernel_body(ctx, tc, nc, mybir, k_d, q_d, wk_d, wv_d, wn8_d, out_d,
                 w_last):
    f32 = mybir.dt.float32
    bf16 = mybir.dt.bfloat16
    Exp = mybir.ActivationFunctionType.Exp

    def pool(name, bufs, space="SBUF"):
        return ctx.enter_context(tc.tile_pool(name=name, bufs=bufs,
                                              space=space))

    consts = pool("consts", 1)
    kpool = pool("kpool", 3)      # raw k slabs [128, 2048] bf16
    t1pool = pool("t1pool", 3)    # pooling intermediate [128, 1024] bf16
    plpool = pool("plpool", 8)    # pooled [128, 512] bf16 per (b, et)
    ktpool = pool("ktpool", 8)    # keysT [128, 512] bf16 per (b, m)
    vpool = pool("vpool", 8)      # values [128, 520] bf16 per (b, ms)
    upool = pool("upool", 16)     # u = exp(scores) [128, 1024] bf16
    outp = pool("outp", 4)        # out [128, 512] f32 per (b, mb)
    rzp = pool("rzp", 16)         # [128, 1] f32 scratch

    ps_proj = pool("ps_proj", 2, "PSUM")  # [128, 512] f32 (1 bank each)
    ps_sc = pool("ps_sc", 2, "PSUM")      # [128, 1024] f32 (2 banks each)
    ps_ctx = pool("ps_ctx", 2, "PSUM")    # [128, 260] f32

    # ---------------- constants / DMA priority queues ----------------
    # act ring (scalar): wn8, wk, k(b0,et1), k(b0,et3), wv, k(b1,et1),
    #                    k(b1,et3), then b0 output
    # sync ring:         k(b0,et0), k(b0,et2), q0, k(b1,et0), k(b1,et2), q1,
    #                    then b1 output
    wn8_sb = consts.tile([128, 4 * H], f32, tag="wn8")
    nc.scalar.dma_start(wn8_sb[:].rearrange("p (m h) -> p m h", m=4),
                        wn8_d[:])
    wk_sb = consts.tile([128, 4 * E], bf16, tag="wk")
    nc.scalar.dma_start(wk_sb[:].rearrange("p (kk e) -> p kk e", kk=4),
                        wk_d[:])
    wv_sb = consts.tile([128, 4 * E], bf16, tag="wv")
    qT = [consts.tile([128, 4 * T], bf16, tag=f"qT{b}", name=f"qT{b}")
          for b in range(B_PER)]
    junk_sb = consts.tile([128, 512], bf16, tag="junk")
    nc.vector.memset(junk_sb[:], 0.0)

    # k DMAs: ring per parity of et; j-slabs contiguous for pooling
    kt_tiles = {}

    def dma_k(b, et):
        kt = kpool.tile([128, 4 * N], bf16, tag="k", name=f"k{b}_{et}")
        eng = nc.sync if et % 2 == 0 else nc.scalar
        eng.dma_start(kt[:].rearrange("p (j n) -> p j n", j=4), k_d[b, et])
        kt_tiles[(b, et)] = kt

    # b0 k + q0 + wv
    dma_k(0, 0)                      # sync
    dma_k(0, 1)                      # act
    dma_k(0, 2)                      # sync
    dma_k(0, 3)                      # act
    nc.sync.dma_start(qT[0][:].rearrange("p (e t) -> p e t", e=4), q_d[0])
    nc.scalar.dma_start(wv_sb[:].rearrange("p (kk e) -> p kk e", kk=4),
                        wv_d[:])
    # b1 k + q1
    dma_k(1, 0)
    dma_k(1, 1)
    dma_k(1, 2)
    dma_k(1, 3)
    nc.sync.dma_start(qT[1][:].rearrange("p (e t) -> p e t", e=4), q_d[1])

    # ---------------- pooling (4-wide sum over j) ----------------
    pooled = {}

    def pool_k(b, et, eng):
        kt = kt_tiles[(b, et)]
        kv = kt[:].rearrange("p (j n) -> p j n", j=4)
        t1 = t1pool.tile([128, 2 * N], bf16, tag="t1", name=f"t1_{b}_{et}")
        t1v = t1[:].rearrange("p (j n) -> p j n", j=2)
        eng.tensor_add(t1v[:, :, :], kv[:, 0:2, :], kv[:, 2:4, :])
        pl = plpool.tile([128, N], bf16, tag="pl", name=f"pl{b}_{et}")
        eng.tensor_add(pl[:], t1v[:, 0, :], t1v[:, 1, :])
        pooled[(b, et)] = pl

    # b0 pooling: all on gpsimd (it cannot read PSUM, so drains go to
    # vector; pooling is gpsimd's only job)
    pool_k(0, 0, nc.gpsimd)
    pool_k(0, 1, nc.gpsimd)
    pool_k(0, 2, nc.gpsimd)
    pool_k(0, 3, nc.gpsimd)

    # ---------------- PE warm-up ----------------
    def junk_block(n):
        ps = ps_proj.tile([128, 512], f32, tag="ps_proj", name="junk")
        for _ in range(n):
            nc.tensor.matmul(ps[:], junk_sb[:, 0:128], junk_sb[:],
                             start=True, stop=True, skip_group_check=True)

    junk_block(N_WARM1)

    # ---------------- projections ----------------
    # keysT[m] = sum_kk wk[kk, m-block]^T @ pooled[kk]  -> [e' 128, n 512]
    keysT = {}

    def kT_chain_mm(b, m, ps, kk, first, last):
        nc.tensor.matmul(
            ps[:],
            wk_sb[:, kk * E + m * 128: kk * E + (m + 1) * 128],
            pooled[(b, kk)][:],
            start=first, stop=last,
        )

    def kT_drain(b, m, ps, eng):
        kt_sb = ktpool.tile([128, N], bf16, tag="keysT", name=f"kT{b}_{m}")
        eng.tensor_copy(kt_sb[:], ps[:])
        keysT[(b, m)] = kt_sb

    # values[ms] = pooled[:, ms-block]^T @ wv -> [n 128, e' 512], stored
    # head-interleaved [n, (h, 65)] with col 64 = Wn (Z column), all
    # pre-scaled by the quadrature mass Wn of that basis row.
    values = {}

    def val_chain(b, ms):
        ps = ps_proj.tile([128, 512], f32, tag="ps_proj", name=f"v{b}_{ms}")
        for i, kk in enumerate((0, 1, 2, 3)):
            nc.tensor.matmul(
                ps[:],
                pooled[(b, kk)][:, ms * 128:(ms + 1) * 128],
                wv_sb[:, kk * E:(kk + 1) * E],
                start=(i == 0), stop=(i == 3),
            )
        v_sb = vpool.tile([128, H * 65], bf16, tag="values",
                          name=f"val{b}_{ms}")
        vv = v_sb[:].rearrange("p (h c) -> p h c", c=65)
        nc.vector.tensor_scalar_mul(
            vv[:, :, 0:64],
            ps[:].rearrange("p (h d) -> p h d", d=64),
            wn8_sb[:, ms * H: ms * H + 1],
        )
        nc.vector.tensor_copy(vv[:, :, 64],
                              wn8_sb[:].rearrange("p (m h) -> p m h",
                                                  m=4)[:, ms, :])
        values[(b, ms)] = v_sb

    # b0 keysT: m0 split around the et2/et3 data arrival, junk in the gap
    ps_m0 = ps_proj.tile([128, 512], f32, tag="ps_proj", name="kT0_0")
    kT_chain_mm(0, 0, ps_m0, 0, True, False)
    kT_chain_mm(0, 0, ps_m0, 1, False, False)
    junk_block(N_WARM2)
    kT_chain_mm(0, 0, ps_m0, 2, False, False)
    kT_chain_mm(0, 0, ps_m0, 3, False, True)
    kT_drain(0, 0, ps_m0, nc.vector)
    for m in (1, 2, 3):
        ps = ps_proj.tile([128, 512], f32, tag="ps_proj", name=f"kT0_{m}")
        for i, kk in enumerate((0, 1, 2, 3)):
            kT_chain_mm(0, m, ps, kk, i == 0, i == 3)
        kT_drain(0, m, ps, nc.vector)
    # b0 values
    for ms in range(4):
        val_chain(0, ms)

    # b1 pooling (gpsimd queue, in data-arrival order)
    pool_k(1, 0, nc.gpsimd)
    pool_k(1, 1, nc.gpsimd)
    pool_k(1, 2, nc.gpsimd)
    pool_k(1, 3, nc.gpsimd)

    # ---------------- scores + exp ----------------
    # scores psum tile [128, 1024] = (h01, nbp, t) for nb pair; row-grouped
    # matmuls: head 2hp+h01 occupies partition rows h01*64..h01*64+64.
    u_tiles = {}

    def score_block(b, hp, ab):
        """4 matmuls filling one [128,1024] psum tile (nb = 2*ab, 2*ab+1),
        then one big exp on the scalar engine."""
        ps = ps_sc.tile([128, 1024], f32, tag="ps_sc", name=f"sc{b}_{hp}_{ab}")
        for nbp in range(2):
            nb = 2 * ab + nbp
            for h01 in range(2):
                nc.tensor.matmul(
                    ps[:, h01 * 512 + nbp * 256: h01 * 512 + nbp * 256 + 256],
                    keysT[(b, hp)][h01 * 64:(h01 + 1) * 64,
                                   nb * 128:(nb + 1) * 128],
                    qT[b][h01 * 64:(h01 + 1) * 64,
                          hp * 256:(hp + 1) * 256],
                    start=True, stop=True,
                    skip_group_check=True,
                )
        u = upool.tile([128, 1024], bf16, tag="u", name=f"u{b}_{hp}_{ab}")
        nc.scalar.activation(u[:], ps[:], Exp)
        u_tiles[(b, hp, ab)] = u

    # ---------------- ctx + normalize ----------------
    out_sbs = {(b, mb): outp.tile([128, E], f32, tag="out",
                                  name=f"out{b}_{mb}")
               for b in range(B_PER) for mb in range(2)}

    def ctx_block(b, hp):
        """4 chains (h01, mb) x 4 nb accumulating into one [128,260] psum
        tile, then normalize on the vector engine."""
        pc = ps_ctx.tile([128, 260], f32, tag="ps_ctx", name=f"ctx{b}_{hp}")
        for h01 in range(2):
            h = hp * 2 + h01
            for mb in range(2):
                c = h01 * 2 + mb
                for nb in range(4):
                    nc.tensor.matmul(
                        pc[:, c * 65:(c + 1) * 65],
                        u_tiles[(b, hp, nb // 2)][
                            :, h01 * 512 + (nb % 2) * 256 + mb * 128:
                            h01 * 512 + (nb % 2) * 256 + (mb + 1) * 128],
                        values[(b, nb)][:, h * 65: h * 65 + 65],
                        start=(nb == 0), stop=(nb == 3),
                        skip_group_check=True,
                    )
        for h01 in range(2):
            h = hp * 2 + h01
            for mb in range(2):
                c = h01 * 2 + mb
                rz = rzp.tile([128, 1], f32, tag="rz")
                nc.vector.tensor_scalar_add(rz[:],
                                            pc[:, c * 65 + 64: c * 65 + 65],
                                            w_last)
                rzi = rzp.tile([128, 1], f32, tag="rzi")
                nc.vector.reciprocal(rzi[:], rz[:])
                nc.vector.tensor_scalar_mul(
                    out_sbs[(b, mb)][:, h * 64:(h + 1) * 64],
                    pc[:, c * 65: c * 65 + 64], rzi[:])

    # -------- segment 2: b0 scores woven with b1 keysT --------
    sc_order = [(hp, ab) for hp in range(4) for ab in range(2)]
    score_block(0, *sc_order[0])
    score_block(0, *sc_order[1])
    # b1 kT m0, m1 (e0, e1 steps)
    ps_b1 = {}
    for m in (0, 1):
        ps_b1[m] = ps_proj.tile([128, 512], f32, tag="ps_proj",
                                name=f"kT1_{m}")
        kT_chain_mm(1, m, ps_b1[m], 0, True, False)
        kT_chain_mm(1, m, ps_b1[m], 1, False, False)
    score_block(0, *sc_order[2])
    score_block(0, *sc_order[3])
    for m in (0, 1):
        kT_chain_mm(1, m, ps_b1[m], 2, False, False)
    score_block(0, *sc_order[4])
    score_block(0, *sc_order[5])
    for m in (0, 1):
        kT_chain_mm(1, m, ps_b1[m], 3, False, True)
        kT_drain(1, m, ps_b1[m], nc.vector)
    score_block(0, *sc_order[6])
    score_block(0, *sc_order[7])
    for m in (2, 3):
        ps = ps_proj.tile([128, 512], f32, tag="ps_proj", name=f"kT1_{m}")
        for i, kk in enumerate((0, 1, 2, 3)):
            kT_chain_mm(1, m, ps, kk, i == 0, i == 3)
        kT_drain(1, m, ps, nc.vector)

    # -------- segment 3: b1 values, then b1 scores woven with b0 ctx ------
    for ms in range(4):
        val_chain(1, ms)

    score_block(1, *sc_order[0])
    score_block(1, *sc_order[1])
    ctx_block(0, 0)
    score_block(1, *sc_order[2])
    score_block(1, *sc_order[3])
    ctx_block(0, 1)
    # b0 out, first column half (heads 0-3)
    for mb in range(2):
        nc.scalar.dma_start(out_d[0, mb * 128:(mb + 1) * 128, 0:256],
                            out_sbs[(0, mb)][:, 0:256])
    score_block(1, *sc_order[4])
    score_block(1, *sc_order[5])
    ctx_block(0, 2)
    score_block(1, *sc_order[6])
    score_block(1, *sc_order[7])
    ctx_block(0, 3)
    for mb in range(2):
        nc.scalar.dma_start(out_d[0, mb * 128:(mb + 1) * 128, 256:512],
                            out_sbs[(0, mb)][:, 256:512])

    # -------- segment 4: b1 ctx --------
    ctx_block(1, 0)
    ctx_block(1, 1)
    for mb in range(2):
        nc.sync.dma_start(out_d[1, mb * 128:(mb + 1) * 128, 0:256],
                          out_sbs[(1, mb)][:, 0:256])
    ctx_block(1, 2)
    ctx_block(1, 3)
    for mb in range(2):
        nc.sync.dma_start(out_d[1, mb * 128:(mb + 1) * 128, 256:512],
                          out_sbs[(1, mb)][:, 256:512])


def _get_program(w_last):
    if "nc" not in _CACHE:
        _CACHE["nc"] = _build_program(w_last)
    return _CACHE["nc"]


def make_in_maps(k, q, Wk, Wv):
    import ml_dtypes
    bf16 = ml_dtypes.bfloat16
    wk_l, wv_l, wn8, w_last = _host_constants(Wk, Wv)
    k16 = np.asarray(k).astype(bf16)
    q16 = np.asarray(q).astype(bf16)
    in_maps = []
    for c in range(N_CORES):
        ks = k16[c * B_PER:(c + 1) * B_PER]          # [2, 512, 2048]
        # -> [b, et, p, j, n]
        kp = np.ascontiguousarray(
            ks.reshape(B_PER, 4, 128, N, 4).transpose(0, 1, 2, 4, 3))
        qs = q16[c * B_PER:(c + 1) * B_PER]          # [2, 256, 512]
        # -> [b, p, eb, t]
        qp = np.ascontiguousarray(
            qs.transpose(0, 2, 1).reshape(B_PER, 4, 128, T)
            .transpose(0, 2, 1, 3))
        in_maps.append({
            "k": kp,
            "q": qp,
            "wk": wk_l,
            "wv": wv_l,
            "wn8": wn8,
        })
    return in_maps, w_last


def kernel(k, q, Wk, Wv):
    from concourse.bass_utils import run_bass_kernel_spmd

    in_maps, w_last = make_in_maps(k, q, Wk, Wv)
    nc = _get_program(w_last)
    res = run_bass_kernel_spmd(nc, in_maps, core_ids=list(range(N_CORES)))
    return np.concatenate([res.results[c]["out"] for c in range(N_CORES)],
                          axis=0)
